# revision 22
# baseline (speedup 1.0000x reference)
# Trainium2 Bass kernel for nn_AttentionBlock (B=2, C=64, D=H=W=16).
#
# reference:
#   q/k/v = 1x1x1 conv (per-voxel channel GEMM) of x  -> [B, C, N], N = 4096
#   attn[b,i,j] = softmax_j(q[:,i] . k[:,j])   (unscaled)
#   out[b,c,i]  = sum_j attn[i,j] v[c,j]
#
# Sharding: 8 cores = 2 batches x 4 query-chunks of 1024 queries each.
# Each core receives its batch's full x (for K/V) plus its query-chunk
# columns, computes K/V for all 4096 keys and flash-style attention for
# its 1024 queries.  No collectives.
#
# All matmuls run in float16 (1 cycle/row on the PE at full clock; f32r
# measured 2x slower because it never engages the HAM clock-gate warmup).
# PSUM accumulation is fp32 throughout, so only operand rounding is f16.
# Measured end-to-end relative error ~7e-4.
#
# Per-core dataflow:
#   x_aug   [65, 4096]  = x_b with a ones row (bias via augmentation)
#   k       [64, 4096]  = WkT_aug.T @ x_aug          (channel-major)
#   q       [64, 1024]  = WqT_aug.T @ xq_aug
#   vT      [128, 32, 65] per key-chunk j: vT[:, j, 0:64] = x_aug.T @ WvT_aug,
#                         vT[:, j, 64] = 1.0  (ones column accumulates l_i)
#   loop over 32 key-chunks j:
#     S^T(j)  [128, 1024] psum = k[:, j].T @ q       (keys on partitions)
#     expT(j) [128, 1024] f16  = Exp(S^T - 12)       (shift is softmax-
#                                invariant and keeps expT in f16 range)
#     out    [65, 1024] psum += vT(j).T @ expT(j)    (row 64 = sum_j expT = l)
#   epilogue: r = 1/l (DVE reciprocal_approx_accurate), broadcast r across
#   partitions (gpsimd), out[c,i] * r[i], DMA out.

import numpy as np

B = 2
C = 64
N = 4096          # tokens per batch
NQ = 1024         # queries per core
NCORES = 8
JC = 128          # key-chunk size
NJ = N // JC      # 32 key chunks
ESHIFT = -12.0    # exp(S + ESHIFT): keeps exp values in f16 range

_CACHE: dict = {}


def _build_bass():
    import concourse.mybir as mybir
    import concourse.tile as tile
    from concourse import bacc

    f32 = mybir.dt.float32
    f16 = mybir.dt.float16

    nc = bacc.Bacc("TRN2")

    xb = nc.dram_tensor("xb", [C, N], f16, kind="ExternalInput")
    xq = nc.dram_tensor("xq", [C, NQ], f16, kind="ExternalInput")
    wqT = nc.dram_tensor("wqT", [C + 1, C], f16, kind="ExternalInput")
    wkT = nc.dram_tensor("wkT", [C + 1, C], f16, kind="ExternalInput")
    wvT = nc.dram_tensor("wvT", [C + 1, C], f16, kind="ExternalInput")
    onesr = nc.dram_tensor("onesr", [1, N], f16, kind="ExternalInput")
    onesc = nc.dram_tensor("onesc", [128, NJ], f16, kind="ExternalInput")
    o = nc.dram_tensor("o", [C, NQ], f32, kind="ExternalOutput")

    with tile.TileContext(nc) as tc:
        with (
            tc.tile_pool(name="singles", bufs=1) as singles,
            tc.tile_pool(name="expp", bufs=3) as expp,
            tc.tile_pool(name="outs", bufs=2) as outs,
            tc.tile_pool(name="pst", bufs=3, space="PSUM") as pst,
            tc.tile_pool(name="pout", bufs=2, space="PSUM") as pout,
        ):
            # ---- load inputs -------------------------------------------------
            # x in 4 column-chunk tiles so projections can start before the
            # whole 512KB transfer lands.
            NXT = 4
            XW = N // NXT
            x_sb = [
                singles.tile([C + 1, XW], f16, name=f"x_sb{t}") for t in range(NXT)
            ]
            xq_sb = singles.tile([C + 1, NQ], f16)
            wq_sb = singles.tile([C + 1, C], f16)
            wk_sb = singles.tile([C + 1, C], f16)
            wv_sb = singles.tile([C + 1, C], f16)
            for t in range(NXT):
                nc.sync.dma_start(out=x_sb[t][0:C, :], in_=xb[:, t * XW : (t + 1) * XW])
                nc.sync.dma_start(out=x_sb[t][C : C + 1, :], in_=onesr[:, 0:XW])
            nc.sync.dma_start(out=xq_sb[0:C, :], in_=xq[:, :])
            nc.sync.dma_start(out=wq_sb[:, :], in_=wqT[:, :])
            nc.sync.dma_start(out=wk_sb[:, :], in_=wkT[:, :])
            nc.sync.dma_start(out=wv_sb[:, :], in_=wvT[:, :])
            nc.sync.dma_start(out=xq_sb[C : C + 1, :], in_=onesr[:, 0:NQ])

            def xs(lo, w):  # x_aug[:, lo:lo+w] within one chunk tile
                t = lo // XW
                assert lo + w <= (t + 1) * XW
                return x_sb[t][:, lo - t * XW : lo - t * XW + w]

            k_sb = singles.tile([C, N], f16)
            q_sb = singles.tile([C, NQ], f16)
            # Replicas on partitions 64:128: odd key-chunks run the S^T matmul
            # in PE row-group 64:127 so its LDWEIGHTS can overlap the previous
            # (row 0:63) matmul via the PE reorder window.
            k2_sb = singles.tile([128, N], f16)
            q2_sb = singles.tile([128, NQ], f16)
            vT_sb = singles.tile([128, NJ, C + 1], f16)
            l_sb = singles.tile([1, NQ], f32)
            lnl_sb = singles.tile([1, NQ], f32)
            rl_sb = singles.tile([1, NQ], f16)
            ones_sb = singles.tile([1, C + 1], f16)
            shift_sb = singles.tile([128, 1], f32)
            nc.vector.memset(shift_sb[:, :], ESHIFT)
            nc.sync.dma_start(out=vT_sb[:, :, C], in_=onesc[:, :])
            nc.sync.dma_start(out=ones_sb[:, :], in_=onesr[:, 0 : C + 1])

            # ---- projections (K, Q channel-major; V transposed) -------------
            for t in range(N // 512):
                ps = pst.tile([128, 1024], f32, tag="st")
                nc.tensor.matmul(ps[0:C, 0:512], wk_sb[:, :], xs(t * 512, 512))
                nc.vector.tensor_copy(k_sb[:, t * 512 : (t + 1) * 512], ps[0:C, 0:512])
            for t in range(NQ // 512):
                ps = pst.tile([128, 1024], f32, tag="st")
                nc.tensor.matmul(
                    ps[0:C, 0:512], wq_sb[:, :], xq_sb[:, t * 512 : (t + 1) * 512]
                )
                nc.vector.tensor_copy(q_sb[:, t * 512 : (t + 1) * 512], ps[0:C, 0:512])
            # vT: 8 key-chunks of [128, 64] per psum tile
            for g in range(NJ // 8):
                ps = pst.tile([128, 1024], f32, tag="st")
                for u in range(8):
                    j = g * 8 + u
                    nc.tensor.matmul(
                        ps[:, u * 64 : (u + 1) * 64], xs(j * JC, JC), wv_sb[:, :]
                    )
                nc.vector.tensor_copy(
                    vT_sb[:, g * 8 : (g + 1) * 8, 0:C],
                    ps[:, 0:512].rearrange("p (u c) -> p u c", c=C),
                )
            # replicate k/q onto partitions 64:128 (SBUF->SBUF DMA)
            nc.sync.dma_start(out=k2_sb[C:128, :], in_=k_sb[:, :])
            nc.sync.dma_start(out=q2_sb[C:128, :], in_=q_sb[:, :])

            # ---- main attention loop over key chunks ------------------------
            op = [
                pout.tile([C + 1, 512], f32, tag="out", name=f"op{i}")
                for i in range(2)
            ]
            for j in range(NJ):
                st = pst.tile([128, 1024], f32, tag="st")
                if j % 2 == 0:
                    kk = k_sb[:, j * JC : (j + 1) * JC]
                    qq = q_sb
                else:
                    kk = k2_sb[C:128, j * JC : (j + 1) * JC]
                    qq = q2_sb[C:128, :]
                for it in range(2):
                    nc.tensor.matmul(
                        st[:, it * 512 : (it + 1) * 512],
                        kk,
                        qq[:, it * 512 : (it + 1) * 512],
                    )
                ex = expp.tile([128, 1024], f16, tag="ex")
                nc.scalar.activation(
                    ex[:, :],
                    st[:, :],
                    mybir.ActivationFunctionType.Exp,
                    bias=shift_sb[:, :],
                )
                for it in range(2):
                    nc.tensor.matmul(
                        op[it][:, :],
                        vT_sb[:, j, :],
                        ex[:, it * 512 : (it + 1) * 512],
                        start=(j == 0),
                        stop=(j == NJ - 1),
                    )

            # ---- epilogue: divide by l (row C of op) ------------------------
            for it in range(2):
                nc.vector.tensor_copy(
                    l_sb[:, it * 512 : (it + 1) * 512], op[it][C : C + 1, :]
                )
            nc.scalar.activation(
                lnl_sb[:, :], l_sb[:, :], mybir.ActivationFunctionType.Ln
            )
            nc.scalar.activation(
                rl_sb[:, :], lnl_sb[:, :], mybir.ActivationFunctionType.Exp, scale=-1.0
            )
            for it in range(2):
                bc = pst.tile([128, 1024], f32, tag="st")
                nc.tensor.matmul(
                    bc[0 : C + 1, 0:512],
                    ones_sb[:, :],
                    rl_sb[:, it * 512 : (it + 1) * 512],
                )
                bc_sb = outs.tile([C, 512], f32, tag="bcs")
                nc.scalar.copy(bc_sb[:, :], bc[0:C, 0:512])
                os = outs.tile([C, 512], f32, tag="os")
                nc.vector.tensor_mul(os[:, :], op[it][0:C, :], bc_sb[:, :])
                nc.sync.dma_start(out=o[:, it * 512 : (it + 1) * 512], in_=os[:, :])

    # The act-table pass picks the first set containing each function, which
    # would load exp_and_others / natural_log alternately (3 x ~1.3us). Hide
    # Exp/Ln from every set except the combined one (set indexes preserved,
    # membership only) so a single natural_log_exp_and_others load serves all.
    import concourse.bacc as bacc_mod
    from concourse.hw_specs import get_activation_tables as _gat

    def _gat_combined(arch):
        t = _gat(arch)
        for name, s in t.items():
            if name != "natural_log_exp_and_others":
                s.discard(mybir.ActivationFunctionType.Exp)
                s.discard(mybir.ActivationFunctionType.Ln)
                s.discard(mybir.ActivationFunctionType.Copy)
        return t

    bacc_mod.get_activation_tables = _gat_combined
    try:
        nc.compile()
    finally:
        bacc_mod.get_activation_tables = _gat
    return nc


def _get_nc():
    if "nc" not in _CACHE:
        _CACHE["nc"] = _build_bass()
    return _CACHE["nc"]


def _in_maps(x, Wq, bq, Wk, bk, Wv, bv):
    x = np.asarray(x, dtype=np.float32)
    xf = np.ascontiguousarray(x.reshape(B, C, N)).astype(np.float16)

    def waug(W, b):
        return np.ascontiguousarray(
            np.vstack(
                [np.asarray(W, np.float32).T, np.asarray(b, np.float32)[None, :]]
            ).astype(np.float16)
        )

    wqT = waug(Wq, bq)
    wkT = waug(Wk, bk)
    wvT = waug(Wv, bv)
    onesr = np.ones((1, N), dtype=np.float16)
    onesc = np.ones((128, NJ), dtype=np.float16)
    maps = []
    for core in range(NCORES):
        b, qc = divmod(core, NCORES // B)
        maps.append(
            {
                "xb": np.ascontiguousarray(xf[b]),
                "xq": np.ascontiguousarray(xf[b][:, qc * NQ : (qc + 1) * NQ]),
                "wqT": wqT,
                "wkT": wkT,
                "wvT": wvT,
                "onesr": onesr,
                "onesc": onesc,
            }
        )
    return maps


def run(inputs: dict, trace: bool = False, tmpdir=None):
    from concourse.bass_utils import run_bass_kernel_spmd

    nc = _get_nc()
    maps = _in_maps(**inputs)
    res = run_bass_kernel_spmd(
        nc, maps, core_ids=list(range(NCORES)), trace=trace, tmpdir=tmpdir
    )
    x = np.asarray(inputs["x"])
    out = np.empty((B, C, N), dtype=np.float32)
    for core in range(NCORES):
        b, qc = divmod(core, NCORES // B)
        out[b][:, qc * NQ : (qc + 1) * NQ] = res.results[core]["o"]
    return out.reshape(x.shape), res


def kernel(**inputs) -> np.ndarray:
    return run(inputs)[0]


# revision 25
# speedup vs baseline: 1.2274x; 1.2274x over previous
# Trainium2 Bass kernel for nn_AttentionBlock (B=2, C=64, D=H=W=16).
#
# reference:
#   q/k/v = 1x1x1 conv (per-voxel channel GEMM) of x  -> [B, C, N], N = 4096
#   attn[b,i,j] = softmax_j(q[:,i] . k[:,j])   (unscaled)
#   out[b,c,i]  = sum_j attn[i,j] v[c,j]
#
# Sharding: 8 cores = 2 batches x 4 query-chunks of 1024 queries each.
# Each core receives its batch's full x (for K/V) plus its query-chunk
# columns, computes K/V for all 4096 keys and flash-style attention for
# its 1024 queries.  No collectives.
#
# All matmuls run in float16 (1 cycle/row on the PE at full clock; f32r
# measured 2x slower because it never engages the HAM clock-gate warmup).
# PSUM accumulation is fp32 throughout, so only operand rounding is f16.
# Measured end-to-end relative error ~7e-4.
#
# Per-core dataflow:
#   x_aug   [65, 4096]  = x_b with a ones row (bias via augmentation)
#   k       [64, 4096]  = WkT_aug.T @ x_aug          (channel-major)
#   q       [64, 1024]  = WqT_aug.T @ xq_aug
#   vT      [128, 32, 65] per key-chunk j: vT[:, j, 0:64] = x_aug.T @ WvT_aug,
#                         vT[:, j, 64] = 1.0  (ones column accumulates l_i)
#   loop over 32 key-chunks j:
#     S^T(j)  [128, 1024] psum = k[:, j].T @ q       (keys on partitions)
#     expT(j) [128, 1024] f16  = Exp(S^T - 12)       (shift is softmax-
#                                invariant and keeps expT in f16 range)
#     out    [65, 1024] psum += vT(j).T @ expT(j)    (row 64 = sum_j expT = l)
#   epilogue: r = 1/l (DVE reciprocal_approx_accurate), broadcast r across
#   partitions (gpsimd), out[c,i] * r[i], DMA out.

import numpy as np

B = 2
C = 64
N = 4096          # tokens per batch
NQ = 1024         # queries per core
NCORES = 8
JC = 128          # key-chunk size
NJ = N // JC      # 32 key chunks
ESHIFT = -12.0    # exp(S + ESHIFT): keeps exp values in f16 range

_CACHE: dict = {}


def _build_bass():
    import concourse.mybir as mybir
    import concourse.tile as tile
    from concourse import bacc

    f32 = mybir.dt.float32
    f16 = mybir.dt.float16

    nc = bacc.Bacc("TRN2")

    xb = nc.dram_tensor("xb", [C, N], f16, kind="ExternalInput")
    xq = nc.dram_tensor("xq", [C, NQ], f16, kind="ExternalInput")
    wqT = nc.dram_tensor("wqT", [C + 1, C], f16, kind="ExternalInput")
    wkT = nc.dram_tensor("wkT", [C + 1, C], f16, kind="ExternalInput")
    wvT = nc.dram_tensor("wvT", [C + 1, C], f16, kind="ExternalInput")
    onesr = nc.dram_tensor("onesr", [1, N], f16, kind="ExternalInput")
    onesc = nc.dram_tensor("onesc", [128, NJ], f16, kind="ExternalInput")
    o = nc.dram_tensor("o", [C, NQ], f32, kind="ExternalOutput")

    with tile.TileContext(nc) as tc:
        with (
            tc.tile_pool(name="singles", bufs=1) as singles,
            tc.tile_pool(name="expp", bufs=3) as expp,
            tc.tile_pool(name="outs", bufs=2) as outs,
            tc.tile_pool(name="pst", bufs=3, space="PSUM") as pst,
            tc.tile_pool(name="pout", bufs=2, space="PSUM") as pout,
        ):
            # ---- load inputs -------------------------------------------------
            # x in 4 column-chunk tiles so projections can start before the
            # whole 512KB transfer lands.
            NXT = 4
            XW = N // NXT
            x_sb = [
                singles.tile([C + 1, XW], f16, name=f"x_sb{t}") for t in range(NXT)
            ]
            xq_sb = singles.tile([C + 1, NQ], f16)
            wq_sb = singles.tile([C + 1, C], f16)
            wk_sb = singles.tile([C + 1, C], f16)
            wv_sb = singles.tile([C + 1, C], f16)
            # Two HWDGE rings (SP + ACT): x data on SP; weights/ones rows on
            # ACT so the first k-proj matmul's inputs land ASAP.
            nc.scalar.dma_start(out=wk_sb[:, :], in_=wkT[:, :])
            nc.scalar.dma_start(out=wq_sb[:, :], in_=wqT[:, :])
            nc.scalar.dma_start(out=wv_sb[:, :], in_=wvT[:, :])
            for t in range(NXT):
                nc.sync.dma_start(out=x_sb[t][0:C, :], in_=xb[:, t * XW : (t + 1) * XW])
                nc.scalar.dma_start(
                    out=x_sb[t][C : C + 1, :], in_=onesr[:, 0:XW]
                )
            nc.sync.dma_start(out=xq_sb[0:C, :], in_=xq[:, :])
            nc.scalar.dma_start(out=xq_sb[C : C + 1, :], in_=onesr[:, 0:NQ])

            def xs(lo, w):  # x_aug[:, lo:lo+w] within one chunk tile
                t = lo // XW
                assert lo + w <= (t + 1) * XW
                return x_sb[t][:, lo - t * XW : lo - t * XW + w]

            k_sb = singles.tile([C, N], f16)
            q_sb = singles.tile([C, NQ], f16)
            vT_sb = singles.tile([128, NJ, C + 1], f16)
            l_sb = singles.tile([1, NQ], f32)
            lnl_sb = singles.tile([1, NQ], f32)
            rl_sb = singles.tile([1, NQ], f16)
            ones_sb = singles.tile([1, C + 1], f16)
            shift_sb = singles.tile([128, 1], f32)
            nc.vector.memset(shift_sb[:, :], ESHIFT)
            nc.scalar.dma_start(out=vT_sb[:, :, C], in_=onesc[:, :])
            nc.scalar.dma_start(out=ones_sb[:, :], in_=onesr[:, 0 : C + 1])

            # ---- projections (K, Q channel-major; V transposed) -------------
            for t in range(N // 512):
                ps = pst.tile([128, 1024], f32, tag="st")
                nc.tensor.matmul(ps[0:C, 0:512], wk_sb[:, :], xs(t * 512, 512))
                nc.vector.tensor_copy(k_sb[:, t * 512 : (t + 1) * 512], ps[0:C, 0:512])
            for t in range(NQ // 512):
                ps = pst.tile([128, 1024], f32, tag="st")
                nc.tensor.matmul(
                    ps[0:C, 0:512], wq_sb[:, :], xq_sb[:, t * 512 : (t + 1) * 512]
                )
                nc.vector.tensor_copy(q_sb[:, t * 512 : (t + 1) * 512], ps[0:C, 0:512])
            # vT: 8 key-chunks of [128, 64] per psum tile
            for g in range(NJ // 8):
                ps = pst.tile([128, 1024], f32, tag="st")
                for u in range(8):
                    j = g * 8 + u
                    nc.tensor.matmul(
                        ps[:, u * 64 : (u + 1) * 64], xs(j * JC, JC), wv_sb[:, :]
                    )
                nc.vector.tensor_copy(
                    vT_sb[:, g * 8 : (g + 1) * 8, 0:C],
                    ps[:, 0:512].rearrange("p (u c) -> p u c", c=C),
                )
            # ---- main attention loop over key chunks ------------------------
            op = [
                pout.tile([C + 1, 512], f32, tag="out", name=f"op{i}")
                for i in range(2)
            ]
            for j in range(NJ):
                st = pst.tile([128, 1024], f32, tag="st")
                for it in range(2):
                    nc.tensor.matmul(
                        st[:, it * 512 : (it + 1) * 512],
                        k_sb[:, j * JC : (j + 1) * JC],
                        q_sb[:, it * 512 : (it + 1) * 512],
                    )
                ex = expp.tile([128, 1024], f16, tag="ex")
                nc.scalar.activation(
                    ex[:, :],
                    st[:, :],
                    mybir.ActivationFunctionType.Exp,
                    bias=shift_sb[:, :],
                )
                for it in range(2):
                    nc.tensor.matmul(
                        op[it][:, :],
                        vT_sb[:, j, :],
                        ex[:, it * 512 : (it + 1) * 512],
                        start=(j == 0),
                        stop=(j == NJ - 1),
                    )

            # ---- epilogue: divide by l (row C of op) ------------------------
            for it in range(2):
                nc.vector.tensor_copy(
                    l_sb[:, it * 512 : (it + 1) * 512], op[it][C : C + 1, :]
                )
            nc.scalar.activation(
                lnl_sb[:, :], l_sb[:, :], mybir.ActivationFunctionType.Ln
            )
            nc.scalar.activation(
                rl_sb[:, :], lnl_sb[:, :], mybir.ActivationFunctionType.Exp, scale=-1.0
            )
            for it in range(2):
                bc = pst.tile([128, 1024], f32, tag="st")
                nc.tensor.matmul(
                    bc[0 : C + 1, 0:512],
                    ones_sb[:, :],
                    rl_sb[:, it * 512 : (it + 1) * 512],
                )
                bc_sb = outs.tile([C, 512], f32, tag="bcs")
                nc.scalar.copy(bc_sb[:, :], bc[0:C, 0:512])
                os = outs.tile([C, 512], f32, tag="os")
                nc.vector.tensor_mul(os[:, :], op[it][0:C, :], bc_sb[:, :])
                nc.sync.dma_start(out=o[:, it * 512 : (it + 1) * 512], in_=os[:, :])

    # The act-table pass picks the first set containing each function, which
    # would load exp_and_others / natural_log alternately (3 x ~1.3us). Hide
    # Exp/Ln from every set except the combined one (set indexes preserved,
    # membership only) so a single natural_log_exp_and_others load serves all.
    import concourse.bacc as bacc_mod
    from concourse.hw_specs import get_activation_tables as _gat

    def _gat_combined(arch):
        t = _gat(arch)
        for name, s in t.items():
            if name != "natural_log_exp_and_others":
                s.discard(mybir.ActivationFunctionType.Exp)
                s.discard(mybir.ActivationFunctionType.Ln)
                s.discard(mybir.ActivationFunctionType.Copy)
        return t

    bacc_mod.get_activation_tables = _gat_combined
    try:
        nc.compile()
    finally:
        bacc_mod.get_activation_tables = _gat
    return nc


def _get_nc():
    if "nc" not in _CACHE:
        _CACHE["nc"] = _build_bass()
    return _CACHE["nc"]


def _in_maps(x, Wq, bq, Wk, bk, Wv, bv):
    x = np.asarray(x, dtype=np.float32)
    xf = np.ascontiguousarray(x.reshape(B, C, N)).astype(np.float16)

    def waug(W, b):
        return np.ascontiguousarray(
            np.vstack(
                [np.asarray(W, np.float32).T, np.asarray(b, np.float32)[None, :]]
            ).astype(np.float16)
        )

    wqT = waug(Wq, bq)
    wkT = waug(Wk, bk)
    wvT = waug(Wv, bv)
    onesr = np.ones((1, N), dtype=np.float16)
    onesc = np.ones((128, NJ), dtype=np.float16)
    maps = []
    for core in range(NCORES):
        b, qc = divmod(core, NCORES // B)
        maps.append(
            {
                "xb": np.ascontiguousarray(xf[b]),
                "xq": np.ascontiguousarray(xf[b][:, qc * NQ : (qc + 1) * NQ]),
                "wqT": wqT,
                "wkT": wkT,
                "wvT": wvT,
                "onesr": onesr,
                "onesc": onesc,
            }
        )
    return maps


def run(inputs: dict, trace: bool = False, tmpdir=None):
    from concourse.bass_utils import run_bass_kernel_spmd

    nc = _get_nc()
    maps = _in_maps(**inputs)
    res = run_bass_kernel_spmd(
        nc, maps, core_ids=list(range(NCORES)), trace=trace, tmpdir=tmpdir
    )
    x = np.asarray(inputs["x"])
    out = np.empty((B, C, N), dtype=np.float32)
    for core in range(NCORES):
        b, qc = divmod(core, NCORES // B)
        out[b][:, qc * NQ : (qc + 1) * NQ] = res.results[core]["o"]
    return out.reshape(x.shape), res


def kernel(**inputs) -> np.ndarray:
    return run(inputs)[0]


# revision 29
# speedup vs baseline: 1.2480x; 1.0168x over previous
# Trainium2 Bass kernel for nn_AttentionBlock (B=2, C=64, D=H=W=16).
#
# reference:
#   q/k/v = 1x1x1 conv (per-voxel channel GEMM) of x  -> [B, C, N], N = 4096
#   attn[b,i,j] = softmax_j(q[:,i] . k[:,j])   (unscaled)
#   out[b,c,i]  = sum_j attn[i,j] v[c,j]
#
# Sharding: 8 cores = 2 batches x 4 query-chunks of 1024 queries each.
# Each core receives its batch's full x (for K/V) plus its query-chunk
# columns, computes K/V for all 4096 keys and flash-style attention for
# its 1024 queries.  No collectives.
#
# All matmuls run in float16 (1 cycle/row on the PE at full clock; f32r
# measured 2x slower because it never engages the HAM clock-gate warmup).
# PSUM accumulation is fp32 throughout, so only operand rounding is f16.
# Measured end-to-end relative error ~7e-4.
#
# Per-core dataflow:
#   x_aug   [65, 4096]  = x_b with a ones row (bias via augmentation)
#   k       [64, 4096]  = WkT_aug.T @ x_aug          (channel-major)
#   q       [64, 1024]  = WqT_aug.T @ xq_aug
#   vT      [128, 32, 65] per key-chunk j: vT[:, j, 0:64] = x_aug.T @ WvT_aug,
#                         vT[:, j, 64] = 1.0  (ones column accumulates l_i)
#   loop over 32 key-chunks j:
#     S^T(j)  [128, 1024] psum = k[:, j].T @ q       (keys on partitions)
#     expT(j) [128, 1024] f16  = Exp(S^T - 12)       (shift is softmax-
#                                invariant and keeps expT in f16 range)
#     out    [65, 1024] psum += vT(j).T @ expT(j)    (row 64 = sum_j expT = l)
#   epilogue: r = 1/l (DVE reciprocal_approx_accurate), broadcast r across
#   partitions (gpsimd), out[c,i] * r[i], DMA out.

import numpy as np

B = 2
C = 64
N = 4096          # tokens per batch
NQ = 1024         # queries per core
NCORES = 8
JC = 128          # key-chunk size
NJ = N // JC      # 32 key chunks
ESHIFT = -12.0    # exp(S + ESHIFT): keeps exp values in f16 range

_CACHE: dict = {}


def _build_bass():
    import concourse.mybir as mybir
    import concourse.tile as tile
    from concourse import bacc

    f32 = mybir.dt.float32
    f16 = mybir.dt.float16

    nc = bacc.Bacc("TRN2")

    # xb/xq carry the ones row (row C) from the host; wall packs
    # [WqT_aug | WkT_aug | WvT_aug+e] = [65, 64+64+65] in one tensor.
    xb = nc.dram_tensor("xb", [C + 1, N], f16, kind="ExternalInput")
    xq = nc.dram_tensor("xq", [C + 1, NQ], f16, kind="ExternalInput")
    wall = nc.dram_tensor("wall", [C + 1, 3 * C + 1], f16, kind="ExternalInput")
    o = nc.dram_tensor("o", [C, NQ], f32, kind="ExternalOutput")

    with tile.TileContext(nc) as tc:
        with (
            tc.tile_pool(name="singles", bufs=1) as singles,
            tc.tile_pool(name="expp", bufs=3) as expp,
            tc.tile_pool(name="outs", bufs=2) as outs,
            tc.tile_pool(name="pst", bufs=3, space="PSUM") as pst,
            tc.tile_pool(name="pout", bufs=2, space="PSUM") as pout,
        ):
            # ---- load inputs -------------------------------------------------
            # x in 2 column-chunk tiles so projections start before the whole
            # transfer lands; weights/xq on the ACT HWDGE ring in parallel.
            NXT = 2
            XW = N // NXT
            x_sb = [
                singles.tile([C + 1, XW], f16, name=f"x_sb{t}") for t in range(NXT)
            ]
            xq_sb = singles.tile([C + 1, NQ], f16)
            w_sb = singles.tile([C + 1, 3 * C + 1], f16)
            nc.scalar.dma_start(out=w_sb[:, :], in_=wall[:, :])
            for t in range(NXT):
                nc.sync.dma_start(out=x_sb[t][:, :], in_=xb[:, t * XW : (t + 1) * XW])
            nc.scalar.dma_start(out=xq_sb[:, :], in_=xq[:, :])
            wq_sb = w_sb[:, 0:C]
            wk_sb = w_sb[:, C : 2 * C]
            wv_sb = w_sb[:, 2 * C : 3 * C + 1]  # [65, 65], col 64 = e_ones

            def xs(lo, w):  # x_aug[:, lo:lo+w] within one chunk tile
                t = lo // XW
                assert lo + w <= (t + 1) * XW
                return x_sb[t][:, lo - t * XW : lo - t * XW + w]

            k_sb = singles.tile([C, N], f16)
            q_sb = singles.tile([C, NQ], f16)
            vT_sb = singles.tile([128, NJ, C + 1], f16)
            l_sb = singles.tile([1, NQ], f32)
            lnl_sb = singles.tile([1, NQ], f32)
            rl_sb = singles.tile([1, NQ], f16)
            shift_sb = singles.tile([128, 1], f32)
            nc.vector.memset(shift_sb[:, :], ESHIFT)
            ones_sb = singles.tile([1, C + 1], f16)
            nc.vector.memset(ones_sb[:, :], 1.0)

            # ---- projections (K, Q channel-major; V transposed) -------------
            for t in range(N // 1024):
                ps = pst.tile([128, 1024], f32, tag="st")
                for h in range(2):
                    nc.tensor.matmul(
                        ps[0:C, h * 512 : (h + 1) * 512],
                        wk_sb,
                        xs(t * 1024 + h * 512, 512),
                    )
                nc.vector.tensor_copy(
                    k_sb[:, t * 1024 : (t + 1) * 1024], ps[0:C, :]
                )
            ps = pst.tile([128, 1024], f32, tag="st", name="psq")
            for h in range(2):
                nc.tensor.matmul(
                    ps[0:C, h * 512 : (h + 1) * 512],
                    wq_sb,
                    xq_sb[:, h * 512 : (h + 1) * 512],
                )
            nc.vector.tensor_copy(q_sb[:, :], ps[0:C, :])
            # vT: 7 key-chunks of [128, 65] per psum tile (65*7 < 512); the
            # ones column comes from wv_sb's e column against x's ones row.
            done = 0
            while done < NJ:
                cnt = min(7, NJ - done)
                ps = pst.tile([128, 1024], f32, tag="st", name="psv")
                for u in range(cnt):
                    j = done + u
                    nc.tensor.matmul(
                        ps[:, u * 65 : (u + 1) * 65], xs(j * JC, JC), wv_sb
                    )
                nc.vector.tensor_copy(
                    vT_sb[:, done : done + cnt, :],
                    ps[:, 0 : cnt * 65].rearrange("p (u c) -> p u c", c=C + 1),
                )
                done += cnt
            # ---- main attention loop over key chunks ------------------------
            op = [
                pout.tile([C + 1, 512], f32, tag="out", name=f"op{i}")
                for i in range(2)
            ]
            for j in range(NJ):
                st = pst.tile([128, 1024], f32, tag="st")
                for it in range(2):
                    nc.tensor.matmul(
                        st[:, it * 512 : (it + 1) * 512],
                        k_sb[:, j * JC : (j + 1) * JC],
                        q_sb[:, it * 512 : (it + 1) * 512],
                    )
                ex = expp.tile([128, 1024], f16, tag="ex")
                nc.scalar.activation(
                    ex[:, :],
                    st[:, :],
                    mybir.ActivationFunctionType.Exp,
                    bias=shift_sb[:, :],
                )
                for it in range(2):
                    nc.tensor.matmul(
                        op[it][:, :],
                        vT_sb[:, j, :],
                        ex[:, it * 512 : (it + 1) * 512],
                        start=(j == 0),
                        stop=(j == NJ - 1),
                    )

            # ---- epilogue: divide by l (row C of op) ------------------------
            for it in range(2):
                nc.vector.tensor_copy(
                    l_sb[:, it * 512 : (it + 1) * 512], op[it][C : C + 1, :]
                )
            nc.scalar.activation(
                lnl_sb[:, :], l_sb[:, :], mybir.ActivationFunctionType.Ln
            )
            nc.scalar.activation(
                rl_sb[:, :], lnl_sb[:, :], mybir.ActivationFunctionType.Exp, scale=-1.0
            )
            for it in range(2):
                bc = pst.tile([128, 1024], f32, tag="st")
                nc.tensor.matmul(
                    bc[0 : C + 1, 0:512],
                    ones_sb[:, :],
                    rl_sb[:, it * 512 : (it + 1) * 512],
                )
                bc_sb = outs.tile([C, 512], f32, tag="bcs")
                nc.scalar.copy(bc_sb[:, :], bc[0:C, 0:512])
                os = outs.tile([C, 512], f32, tag="os")
                nc.vector.tensor_mul(os[:, :], op[it][0:C, :], bc_sb[:, :])
                nc.sync.dma_start(out=o[:, it * 512 : (it + 1) * 512], in_=os[:, :])

    # The act-table pass picks the first set containing each function, which
    # would load exp_and_others / natural_log alternately (3 x ~1.3us). Hide
    # Exp/Ln from every set except the combined one (set indexes preserved,
    # membership only) so a single natural_log_exp_and_others load serves all.
    import concourse.bacc as bacc_mod
    from concourse.hw_specs import get_activation_tables as _gat

    def _gat_combined(arch):
        t = _gat(arch)
        for name, s in t.items():
            if name != "natural_log_exp_and_others":
                s.discard(mybir.ActivationFunctionType.Exp)
                s.discard(mybir.ActivationFunctionType.Ln)
                s.discard(mybir.ActivationFunctionType.Copy)
        return t

    bacc_mod.get_activation_tables = _gat_combined
    try:
        nc.compile()
    finally:
        bacc_mod.get_activation_tables = _gat
    return nc


def _get_nc():
    if "nc" not in _CACHE:
        _CACHE["nc"] = _build_bass()
    return _CACHE["nc"]


def _in_maps(x, Wq, bq, Wk, bk, Wv, bv):
    x = np.asarray(x, dtype=np.float32)
    ones = np.ones((1, N), dtype=np.float32)
    xf = np.concatenate([x.reshape(B, C, N), np.broadcast_to(ones, (B, 1, N))], axis=1)
    xf = np.ascontiguousarray(xf).astype(np.float16)  # [B, C+1, N] with ones row

    def waug(W, b):
        return np.vstack(
            [np.asarray(W, np.float32).T, np.asarray(b, np.float32)[None, :]]
        )

    e = np.zeros((C + 1, 1), dtype=np.float32)
    e[C, 0] = 1.0
    wall = np.ascontiguousarray(
        np.hstack([waug(Wq, bq), waug(Wk, bk), waug(Wv, bv), e]).astype(np.float16)
    )
    maps = []
    for core in range(NCORES):
        b, qc = divmod(core, NCORES // B)
        maps.append(
            {
                "xb": np.ascontiguousarray(xf[b]),
                "xq": np.ascontiguousarray(xf[b][:, qc * NQ : (qc + 1) * NQ]),
                "wall": wall,
            }
        )
    return maps


def run(inputs: dict, trace: bool = False, tmpdir=None):
    from concourse.bass_utils import run_bass_kernel_spmd

    nc = _get_nc()
    maps = _in_maps(**inputs)
    res = run_bass_kernel_spmd(
        nc, maps, core_ids=list(range(NCORES)), trace=trace, tmpdir=tmpdir
    )
    x = np.asarray(inputs["x"])
    out = np.empty((B, C, N), dtype=np.float32)
    for core in range(NCORES):
        b, qc = divmod(core, NCORES // B)
        out[b][:, qc * NQ : (qc + 1) * NQ] = res.results[core]["o"]
    return out.reshape(x.shape), res


def kernel(**inputs) -> np.ndarray:
    return run(inputs)[0]
